# revision 33
# baseline (speedup 1.0000x reference)
"""Clustered-attention Trainium2 kernel (Bass/Tile), 8-core SPMD.

Problem (per batch b, variable k, with L=512, V=32, D=64, C=8 clusters):
    S   = sum_v key[b,:,v,:]                  # (L, D) shared key-sum
    sc  = query[b,:,k,:] @ S.T / sqrt(D)      # (L, L)
    sc  = where(label[i]==label[j], sc, -inf)
    out = softmax(sc, -1) @ value[b,:,k,:]

Sharding: 8 cores = 4 batches x 2 halves of the v axis (16 heads/core).

Key ideas (all FLOPs on device; host only reshapes/casts/permutes):
  - Host sorts each batch by label. Every cluster (<=~90 rows, hard
    bound 128 assumed) then lies within a 128-row window, so only chunk
    pairs |ci-cj| <= 1 interact: 10 of 16 (i,j) 128-chunk pairs are
    computed (62.5% of dense work on PE and ScalarE).
  - The cluster mask is folded into the scores matmul: the contraction
    dim is extended by 8 one-hot label rows scaled 8*B (B=96) on the
    lhsT side and 1.0 on the rhs side, so z = q.s + 8B*[same cluster];
    exp(z/8 - B) is exp(q.s/8) for same-cluster pairs and <= e^-61
    otherwise (vs real terms >= e^-35) -- an exact -inf mask to ~1e-10,
    with zero per-element masking cost. This also makes the sorted
    windows exact: any cross-cluster overlap inside a window is zeroed.
  - keysum via fp16 tree adds on DVE over a host-interleaved
    [128p, 4c, 2048] key layout (fat 16KB DMA descriptors), then
    PE-transposed per chunk into [S^T; 8B*onehot] lhsT tiles.
  - scores^T windows [128j, w<=384] on PE (fp16, fp32 PSUM), exp on
    ScalarE (bf16 out, zero-gap saturated -- the bottleneck engine),
    then A@V accumulates E^T as lhsT so the output lands directly as
    [i, d | denom] in PSUM; the softmax denominator comes from a ones
    column appended to V by the host. One reciprocal + one 0-stride
    broadcast multiply per head normalize it.
  - Prologue tricks: exp activation-table preloaded via a dummy op, 10
    junk matmuls warm the PE HAM clock gate to 2.4 GHz, DMA issue (~0.6
    us/dma_start, serial per sequencer) kept to few fat instructions
    split across the GpSimd and SP issue paths, A@V software-pipelined
    one head behind the scores so the PE never waits on the live exp.

Measured on trn2 (8 cores, NTFF profile): ~67 us end-to-end per core,
scale-relative absmax error ~4.7e-3 (fp16/bf16 quantization).
"""

import numpy as np

import concourse.bass as bass
import concourse.tile as tile
from concourse import mybir
from concourse.masks import make_identity
from concourse.tile import TileContext, ScopedClock

B, L, V, D = 4, 512, 32, 64
NC = 8  # cores
VH = V // 2  # heads (variables) per core
NJ = L // 128  # j/i chunks
HG_OUT = 2
BIAS = 96.0  # mask bias (see module docstring)
F32 = mybir.dt.float32
F16 = mybir.dt.float16
BF16 = mybir.dt.bfloat16

PROFILE = False  # set True from a harness to enable NTFF tracing
LAST_RESULT = None  # BassKernelResults of the most recent run

_PATCHED = False


def _patch_tile_drain():
    """Walrus on this image rejects multiple sync-waits on one instruction
    ("Too many sync wait commands"). Legalize by splitting surplus waits
    onto NoOp instructions inserted just before, on the same engine —
    identical semantics (the engine stalls at each wait in order)."""
    global _PATCHED
    if _PATCHED:
        return
    _PATCHED = True

    _orig_add = TileContext._add_instruction

    def _add_instruction(self, inst):
        si = getattr(inst, "sync_info", None)
        if (
            si is not None
            and si.on_wait
            and len(si.on_wait) > 1
            and inst.engine != mybir.EngineType.Unassigned
        ):
            waits = list(si.on_wait)
            for w in waits[:-1]:
                nop = mybir.InstNoOp(name=self.nc.get_next_instruction_name())
                nop.engine = inst.engine
                nop.sync_info = mybir.SyncInfo(on_wait=[w], on_update=[])
                _orig_add(self, nop)
            inst.sync_info = mybir.SyncInfo(
                on_wait=[waits[-1]], on_update=list(si.on_update or [])
            )
        _orig_add(self, inst)

    TileContext._add_instruction = _add_instruction

    def _drain_and_barrier(self, tick_clock, wait_clock):
        nc = self.nc
        drain_inst = nc.sync.drain()
        wait_clock.add_sem_waits(
            drain_inst.ins, ScopedClock({None: tick_clock.global_clock})
        )
        si = drain_inst.ins.sync_info
        if si is not None and si.on_wait and len(si.on_wait) > 1:
            waits = list(si.on_wait)
            drain_inst.ins.sync_info = mybir.SyncInfo(
                on_wait=waits[:1], on_update=list(si.on_update or [])
            )
            for i in range(1, len(waits)):
                nop = nc.sync.nop(nofuse=True, hint=f"drain_split_{i}")
                nop.ins.sync_info = mybir.SyncInfo(on_wait=[waits[i]], on_update=[])
        nc.all_engine_barrier()
        assert self.sems is not None
        popped = nc._tile_sem_poison_stack.pop()
        assert popped is self._sem_poison
        nc.clear_and_free_semaphores(list(self.sems.allocated().values()))
        nc.all_engine_barrier()

    TileContext._drain_and_barrier = _drain_and_barrier


def _tree_reduce_v(eng, pool, kc, tag):
    """Sum kc [128, V*D] over the v axis -> [128, D] via contiguous
    halving adds on the given engine (v-major layout: halves contiguous)."""
    cur = kc
    width = V * D
    while width > D:
        width //= 2
        nxt = pool.tile([128, width], kc.dtype, tag=f"red_{width}")
        eng.tensor_tensor(
            out=nxt, in0=cur[:, 0:width], in1=cur[:, width : 2 * width],
            op=mybir.AluOpType.add,
        )
        cur = nxt
    return cur


def _dma_split(nc, out_ap, in_ap, parts):
    """Issue `parts` dma_starts over free-dim slices so the transfer
    spreads across DMA queues instead of serializing on one."""
    w = out_ap.shape[-1]
    step = w // parts
    for i in range(parts):
        sl = slice(i * step, (i + 1) * step) if i < parts - 1 else slice(i * step, w)
        nc.sync.dma_start(out=out_ap[..., sl], in_=in_ap[..., sl])


def _build_nc():
    nc = bass.Bass("TRN2", target_bir_lowering=False, debug=False)

    # All bulk inputs arrive in 16-bit, host-prepared layouts chosen for
    # fat DMA descriptors (per-partition-contiguous rows) and are already
    # PERMUTED so labels are sorted per batch: every cluster then lives
    # inside a 128-row window, and only chunk pairs |ci-cj|<=1 interact.
    q_t = nc.dram_tensor("q_t", [D, VH * L], F16, kind="ExternalInput").ap()
    ka_in = nc.dram_tensor("ka", [128, NJ * (V // 2) * D], F16,
                           kind="ExternalInput").ap()
    kb_in = nc.dram_tensor("kb", [128, NJ * (V // 2) * D], F16,
                           kind="ExternalInput").ap()
    v_in = nc.dram_tensor("v", [128, NJ, VH, D + 2], BF16, kind="ExternalInput").ap()
    lab = nc.dram_tensor("lab", [1, L], F32, kind="ExternalInput").ap()
    iota8 = nc.dram_tensor("iota8", [8, 1], F32, kind="ExternalInput").ap()
    # output in sorted order, grouped: [g, p, si, hh, d] (4 groups x 4 heads)
    o_out = nc.dram_tensor("o", [8, 128, NJ * 2 * D], F32, kind="ExternalOutput").ap()

    # i-chunk windows per j-chunk: chunks [lo, hi] inclusive
    WIN = [(max(jc - 1, 0), min(jc + 1, NJ - 1)) for jc in range(NJ)]

    with TileContext(nc) as tc:
        with (
            tc.tile_pool(name="singles", bufs=1) as singles,
            tc.tile_pool(name="redpool", bufs=2) as redpool,
            tc.tile_pool(name="epool", bufs=10) as epool,
            tc.tile_pool(name="rpool", bufs=3) as rpool,
            tc.tile_pool(name="ps_score", bufs=4, space="PSUM") as ps_score,
            tc.tile_pool(name="ps_u", bufs=2, space="PSUM") as ps_u,
            tc.tile_pool(name="ps_t", bufs=1, space="PSUM") as ps_t,
        ):
            # ---- constants first (no DMA deps; must not queue behind
            # DMA issue on any sequencer) ----
            identity = singles.tile([128, 128], F16)
            make_identity(nc, identity)
            negb = singles.tile([128, 1], F32)
            nc.vector.memset(negb, -BIAS)
            dummy = singles.tile([128, 1], F32)
            nc.scalar.activation(dummy, negb,
                                 mybir.ActivationFunctionType.Exp)
            junk = singles.tile([128, L], F16)
            nc.vector.memset(junk, 1.0)

            # ---- bulk input DMAs: few instructions (issue costs ~0.6us
            # each), fat descriptors, earliest-needed first, spread over
            # both the GpSimd (opens earlier) and SP issue paths ----
            lab_sb = singles.tile([8, L], F32)
            lab_bcast = bass.AP(tensor=lab.tensor, offset=lab.offset,
                                ap=[[0, 8]] + list(lab.ap[1:]))
            nc.gpsimd.dma_start(out=lab_sb, in_=lab_bcast)
            iota_sb = singles.tile([8, 1], F32)
            nc.gpsimd.dma_start(out=iota_sb, in_=iota8)
            qtb = singles.tile([D + 8, VH * L], F16)
            nc.gpsimd.dma_start(out=qtb[0:32, :], in_=q_t[0:32, :])
            nc.gpsimd.dma_start(out=qtb[32:D, :], in_=q_t[32:D, :])

            # key v-halves over 8 HWDGE queues (SP) — fastest measured
            # path for the 2MB key; k_a's tree overlaps k_b's transfer
            kc_a = singles.tile([128, NJ * (V // 2) * D], F16)
            kc_b = singles.tile([128, NJ * (V // 2) * D], F16)
            for i in range(4):
                nc.sync.dma_start(out=kc_a[i * 32 : (i + 1) * 32],
                                  in_=ka_in[i * 32 : (i + 1) * 32])
            for i in range(4):
                nc.sync.dma_start(out=kc_b[i * 32 : (i + 1) * 32],
                                  in_=kb_in[i * 32 : (i + 1) * 32])
            vcast = singles.tile([128, NJ, VH, D + 2], BF16)
            nc.gpsimd.dma_start(out=vcast[0:64], in_=v_in[0:64])
            nc.gpsimd.dma_start(out=vcast[64:128], in_=v_in[64:128])

            # one-hot label rows (device-computed from sorted labels)
            onehot = singles.tile([8, L], F32)
            nc.vector.tensor_scalar(onehot, lab_sb, iota_sb, None,
                                    op0=mybir.AluOpType.is_equal)
            oh16 = singles.tile([8, L], F16)
            nc.vector.tensor_copy(oh16, onehot)
            # replicate into the query tile rows (one 0-stride DMA)
            oh_rep = bass.AP(tensor=oh16.tensor, offset=oh16.offset,
                             ap=[list(oh16.ap[0]), [0, VH], list(oh16.ap[1])])
            nc.gpsimd.dma_start(
                out=qtb[D : D + 8, :].rearrange("p (h l) -> p h l", h=VH, l=L),
                in_=oh_rep,
            )

            # stb tiles: one-hot rows written by DVE, S^T rows by keysum
            stbs = {}
            for jc in range(NJ):
                stbs[jc] = singles.tile([D + 8, 128], F16, tag=f"stb{jc}",
                                        name=f"stb{jc}")
                nc.vector.tensor_scalar_mul(
                    stbs[jc][D : D + 8, :],
                    onehot[:, jc * 128 : (jc + 1) * 128], 8.0 * BIAS,
                )

            # ---- PE warmup so HAM reaches 2.4 GHz before the pipeline ----
            for w in range(10):
                wps = ps_u.tile([128, L], F32, tag="warm", name=f"warm{w}", bufs=1)
                nc.tensor.matmul(wps, lhsT=identity, rhs=junk,
                                 start=True, stop=True)

            # ---- keysum: each v-half reduced independently (the first
            # tree runs while the second half is still in flight), then
            # one final add ----
            def _half_tree(kc, tag_):
                width = (V // 2) * D
                cur = kc.rearrange("p (c w) -> p c w", c=NJ, w=width)
                while width > D:
                    width //= 2
                    nxt = redpool.tile([128, NJ, width], F16,
                                       tag=f"red{tag_}{width}",
                                       name=f"red{tag_}{width}")
                    nc.vector.tensor_tensor(
                        out=nxt, in0=cur[:, :, 0:width],
                        in1=cur[:, :, width : 2 * width],
                        op=mybir.AluOpType.add,
                    )
                    cur = nxt
                return cur

            s_a = _half_tree(kc_a, "a")
            s_b = _half_tree(kc_b, "b")
            cur = redpool.tile([128, NJ, D], F16, tag="redsum", name="redsum")
            nc.vector.tensor_tensor(out=cur, in0=s_a, in1=s_b,
                                    op=mybir.AluOpType.add)

            def _make_stb(jc):
                st_ps = ps_t.tile([D, 128], F16, tag="st_ps", name=f"st{jc}")
                nc.tensor.transpose(st_ps, cur[:, jc, :], identity)
                nc.vector.tensor_copy(stbs[jc][0:D, :], st_ps)

            HG = 2  # output group = 2 heads
            oc_tiles = [
                singles.tile([128, NJ, HG, D], F32, tag=f"oc{g}", name=f"oc{g}")
                for g in range(VH // HG)
            ]

            def _head_scores(h, jc):
                lo, hi = WIN[jc]
                w = 128 * (hi - lo + 1)
                ps = ps_score.tile([128, L], F32, tag="ps", name=f"ps{h}_{jc}")
                nc.tensor.matmul(
                    ps[:, 0:w], lhsT=stbs[jc],
                    rhs=qtb[:, h * L + 128 * lo : h * L + 128 * lo + w],
                    start=True, stop=True,
                )
                e_t = epool.tile([128, 3 * 128], BF16, tag="et",
                                 name=f"et{h}_{jc}")
                nc.scalar.activation(
                    e_t[:, 0:w], ps[:, 0:w], mybir.ActivationFunctionType.Exp,
                    bias=negb, scale=1.0 / 8.0,
                )
                return e_t

            def _head_tail(h, e_tiles):
                oc = oc_tiles[h // HG]
                hh = h % HG
                psu = ps_u.tile([128, NJ, D + 1], F32, tag="psu", name=f"psu{h}")
                for si in range(NJ):
                    jcs = [jc for jc in range(NJ)
                           if WIN[jc][0] <= si <= WIN[jc][1]]
                    for idx, jc in enumerate(jcs):
                        off = 128 * (si - WIN[jc][0])
                        nc.tensor.matmul(
                            psu[:, si, :],
                            lhsT=e_tiles[jc][:, off : off + 128],
                            rhs=vcast[:, jc, h, 0 : D + 1],
                            start=(idx == 0), stop=(idx == len(jcs) - 1),
                        )
                rinv = rpool.tile([128, NJ], F32, tag="rinv", name=f"rinv{h}")
                nc.vector.reciprocal(rinv, psu[:, :, D])
                rinv_b = bass.AP(
                    tensor=rinv.tensor, offset=rinv.offset,
                    ap=[list(rinv.ap[0]), list(rinv.ap[1]), [0, D]],
                )
                nc.vector.tensor_tensor(
                    out=oc[:, :, hh, :], in0=psu[:, :, 0:D], in1=rinv_b,
                    op=mybir.AluOpType.mult,
                )
                if hh == HG - 1:
                    g = h // HG
                    flat = oc.rearrange("p a b d -> p (a b d)")
                    nc.sync.dma_start(out=o_out[g, 0:64], in_=flat[0:64])
                    nc.sync.dma_start(out=o_out[g, 64:128], in_=flat[64:128])

            # ---- head pipeline. stb0/1 come up first; heads 0-1 run
            # their jc0/jc1 scores before stb2/3 exist so the PE stream
            # matches data arrival. A@V runs one head behind the scores
            # so the PE never waits on the live head's exp. ----
            _make_stb(0)
            _make_stb(1)
            e0 = {0: _head_scores(0, 0), 1: _head_scores(0, 1)}
            e1 = {0: _head_scores(1, 0), 1: _head_scores(1, 1)}
            _make_stb(2)
            _make_stb(3)
            e0[2] = _head_scores(0, 2)
            e0[3] = _head_scores(0, 3)
            e1[2] = _head_scores(1, 2)
            e1[3] = _head_scores(1, 3)
            _head_tail(0, [e0[j] for j in range(NJ)])
            prev = (1, [e1[j] for j in range(NJ)])
            for h in range(2, VH):
                e_tiles = [_head_scores(h, jc) for jc in range(NJ)]
                _head_tail(*prev)
                prev = (h, e_tiles)
            _head_tail(*prev)
    return nc


_NC_CACHE = None


def _get_nc():
    global _NC_CACHE
    if _NC_CACHE is None:
        _patch_tile_drain()
        _NC_CACHE = _build_nc()
    return _NC_CACHE


def kernel(query, key, value, label_arr):
    """Full inputs (B,L,V,D)/(B,L) -> full output (B,L,V,D)."""
    global LAST_RESULT
    import ml_dtypes
    from concourse.bass_utils import run_bass_kernel_spmd

    query = np.asarray(query, dtype=np.float32)
    key = np.asarray(key, dtype=np.float32)
    value = np.asarray(value, dtype=np.float32)
    labels = np.asarray(label_arr)
    iota = np.arange(8, dtype=np.float32).reshape(8, 1)

    in_maps, perms = [], []
    for c in range(NC):
        b, v0 = c // 2, (c % 2) * VH
        perm = np.argsort(labels[b], kind="stable")
        perms.append(perm)
        qp = query[b][perm][:, v0 : v0 + VH, :]      # (L, VH, D) sorted
        kp = key[b][perm]                            # (L, V, D) sorted
        vp_ = value[b][perm][:, v0 : v0 + VH, :]
        labp = labels[b][perm].astype(np.float32)

        vp = np.zeros((L, VH, D + 2), dtype=ml_dtypes.bfloat16)
        vp[:, :, 0:D] = vp_.astype(ml_dtypes.bfloat16)
        vp[:, :, D] = 1.0
        in_maps.append({
            "q_t": np.ascontiguousarray(qp.transpose(2, 1, 0))
                .astype(np.float16).reshape(D, VH * L),
            "ka": np.ascontiguousarray(
                kp.reshape(NJ, 128, V, D)[:, :, : V // 2]
                .transpose(1, 0, 2, 3)
            ).astype(np.float16).reshape(128, NJ * (V // 2) * D),
            "kb": np.ascontiguousarray(
                kp.reshape(NJ, 128, V, D)[:, :, V // 2 :]
                .transpose(1, 0, 2, 3)
            ).astype(np.float16).reshape(128, NJ * (V // 2) * D),
            "v": np.ascontiguousarray(
                vp.reshape(NJ, 128, VH, D + 2).transpose(1, 0, 2, 3)
            ),
            "lab": labp.reshape(1, L).copy(),
            "iota8": iota,
        })

    nc = _get_nc()
    kwargs = {}
    if PROFILE:
        kwargs["trace"] = True
    res = run_bass_kernel_spmd(nc, in_maps, list(range(NC)), **kwargs)
    LAST_RESULT = res

    out = np.empty((B, L, V, D), dtype=np.float32)
    for c in range(NC):
        b, v0 = c // 2, (c % 2) * VH
        # o: [g, p, (si hh d)] -> sorted (L, VH, D), then inverse-permute
        o = res.results[c]["o"].reshape(8, 128, NJ, HG_OUT, D)
        o_sorted = o.transpose(2, 1, 0, 3, 4).reshape(L, VH, D)
        out[b][perms[c], v0 : v0 + VH, :] = o_sorted
    return out


# revision 34
# speedup vs baseline: 1.0336x; 1.0336x over previous
"""Clustered-attention Trainium2 kernel (Bass/Tile), 8-core SPMD.

Problem (per batch b, variable k, with L=512, V=32, D=64, C=8 clusters):
    S   = sum_v key[b,:,v,:]                  # (L, D) shared key-sum
    sc  = query[b,:,k,:] @ S.T / sqrt(D)      # (L, L)
    sc  = where(label[i]==label[j], sc, -inf)
    out = softmax(sc, -1) @ value[b,:,k,:]

Sharding: 8 cores = 4 batches x 2 halves of the v axis (16 heads/core).

Key ideas (all FLOPs on device; host only reshapes/casts/permutes):
  - Host sorts each batch by label. Every cluster (<=~90 rows, hard
    bound 128 assumed) then lies within a 128-row window, so only chunk
    pairs |ci-cj| <= 1 interact: 10 of 16 (i,j) 128-chunk pairs are
    computed (62.5% of dense work on PE and ScalarE).
  - The cluster mask is folded into the scores matmul: the contraction
    dim is extended by 8 one-hot label rows scaled 8*B (B=96) on the
    lhsT side and 1.0 on the rhs side, so z = q.s + 8B*[same cluster];
    exp(z/8 - B) is exp(q.s/8) for same-cluster pairs and <= e^-61
    otherwise (vs real terms >= e^-35) -- an exact -inf mask to ~1e-10,
    with zero per-element masking cost. This also makes the sorted
    windows exact: any cross-cluster overlap inside a window is zeroed.
  - keysum via fp16 tree adds on DVE over a host-interleaved
    [128p, 4c, 2048] key layout (fat 16KB DMA descriptors), then
    PE-transposed per chunk into [S^T; 8B*onehot] lhsT tiles.
  - scores^T windows [128j, w<=384] on PE (fp16, fp32 PSUM), exp on
    ScalarE (bf16 out, zero-gap saturated -- the bottleneck engine),
    then A@V accumulates E^T as lhsT so the output lands directly as
    [i, d | denom] in PSUM; the softmax denominator comes from a ones
    column appended to V by the host. One reciprocal + one 0-stride
    broadcast multiply per head normalize it.
  - Prologue tricks: exp activation-table preloaded via a dummy op, 10
    junk matmuls warm the PE HAM clock gate to 2.4 GHz, DMA issue (~0.6
    us/dma_start, serial per sequencer) kept to few fat instructions
    split across the GpSimd and SP issue paths, A@V software-pipelined
    one head behind the scores so the PE never waits on the live exp.

Measured on trn2 (8 cores, NTFF profile): ~67 us end-to-end per core,
scale-relative absmax error ~4.7e-3 (fp16/bf16 quantization).
"""

import numpy as np

import concourse.bass as bass
import concourse.tile as tile
from concourse import mybir
from concourse.masks import make_identity
from concourse.tile import TileContext, ScopedClock

B, L, V, D = 4, 512, 32, 64
NC = 8  # cores
VH = V // 2  # heads (variables) per core
NJ = L // 128  # j/i chunks
HG_OUT = 2
BIAS = 96.0  # mask bias (see module docstring)
F32 = mybir.dt.float32
F16 = mybir.dt.float16
BF16 = mybir.dt.bfloat16

PROFILE = False  # set True from a harness to enable NTFF tracing
LAST_RESULT = None  # BassKernelResults of the most recent run

_PATCHED = False


def _patch_tile_drain():
    """Walrus on this image rejects multiple sync-waits on one instruction
    ("Too many sync wait commands"). Legalize by splitting surplus waits
    onto NoOp instructions inserted just before, on the same engine —
    identical semantics (the engine stalls at each wait in order)."""
    global _PATCHED
    if _PATCHED:
        return
    _PATCHED = True

    _orig_add = TileContext._add_instruction

    def _add_instruction(self, inst):
        si = getattr(inst, "sync_info", None)
        if (
            si is not None
            and si.on_wait
            and len(si.on_wait) > 1
            and inst.engine != mybir.EngineType.Unassigned
        ):
            waits = list(si.on_wait)
            for w in waits[:-1]:
                nop = mybir.InstNoOp(name=self.nc.get_next_instruction_name())
                nop.engine = inst.engine
                nop.sync_info = mybir.SyncInfo(on_wait=[w], on_update=[])
                _orig_add(self, nop)
            inst.sync_info = mybir.SyncInfo(
                on_wait=[waits[-1]], on_update=list(si.on_update or [])
            )
        _orig_add(self, inst)

    TileContext._add_instruction = _add_instruction

    def _drain_and_barrier(self, tick_clock, wait_clock):
        nc = self.nc
        drain_inst = nc.sync.drain()
        wait_clock.add_sem_waits(
            drain_inst.ins, ScopedClock({None: tick_clock.global_clock})
        )
        si = drain_inst.ins.sync_info
        if si is not None and si.on_wait and len(si.on_wait) > 1:
            waits = list(si.on_wait)
            drain_inst.ins.sync_info = mybir.SyncInfo(
                on_wait=waits[:1], on_update=list(si.on_update or [])
            )
            for i in range(1, len(waits)):
                nop = nc.sync.nop(nofuse=True, hint=f"drain_split_{i}")
                nop.ins.sync_info = mybir.SyncInfo(on_wait=[waits[i]], on_update=[])
        nc.all_engine_barrier()
        assert self.sems is not None
        popped = nc._tile_sem_poison_stack.pop()
        assert popped is self._sem_poison
        nc.clear_and_free_semaphores(list(self.sems.allocated().values()))
        nc.all_engine_barrier()

    TileContext._drain_and_barrier = _drain_and_barrier


def _tree_reduce_v(eng, pool, kc, tag):
    """Sum kc [128, V*D] over the v axis -> [128, D] via contiguous
    halving adds on the given engine (v-major layout: halves contiguous)."""
    cur = kc
    width = V * D
    while width > D:
        width //= 2
        nxt = pool.tile([128, width], kc.dtype, tag=f"red_{width}")
        eng.tensor_tensor(
            out=nxt, in0=cur[:, 0:width], in1=cur[:, width : 2 * width],
            op=mybir.AluOpType.add,
        )
        cur = nxt
    return cur


def _dma_split(nc, out_ap, in_ap, parts):
    """Issue `parts` dma_starts over free-dim slices so the transfer
    spreads across DMA queues instead of serializing on one."""
    w = out_ap.shape[-1]
    step = w // parts
    for i in range(parts):
        sl = slice(i * step, (i + 1) * step) if i < parts - 1 else slice(i * step, w)
        nc.sync.dma_start(out=out_ap[..., sl], in_=in_ap[..., sl])


def _build_nc():
    nc = bass.Bass("TRN2", target_bir_lowering=False, debug=False)

    # All bulk inputs arrive in 16-bit, host-prepared layouts chosen for
    # fat DMA descriptors (per-partition-contiguous rows) and are already
    # PERMUTED so labels are sorted per batch: every cluster then lives
    # inside a 128-row window, and only chunk pairs |ci-cj|<=1 interact.
    q_t = nc.dram_tensor("q_t", [D, VH * L], F16, kind="ExternalInput").ap()
    ka_in = nc.dram_tensor("ka", [128, NJ * (V // 2) * D], F16,
                           kind="ExternalInput").ap()
    kb_in = nc.dram_tensor("kb", [128, NJ * (V // 2) * D], F16,
                           kind="ExternalInput").ap()
    v_in = nc.dram_tensor("v", [128, NJ, VH, D + 2], BF16, kind="ExternalInput").ap()
    lab = nc.dram_tensor("lab", [1, L], F32, kind="ExternalInput").ap()
    iota8 = nc.dram_tensor("iota8", [8, 1], F32, kind="ExternalInput").ap()
    # output in sorted order, grouped: [g, p, si, hh, d] (4 groups x 4 heads)
    o_out = nc.dram_tensor("o", [8, 128, NJ * 2 * D], F32, kind="ExternalOutput").ap()

    # i-chunk windows per j-chunk: chunks [lo, hi] inclusive
    WIN = [(max(jc - 1, 0), min(jc + 1, NJ - 1)) for jc in range(NJ)]

    with TileContext(nc) as tc:
        with (
            tc.tile_pool(name="singles", bufs=1) as singles,
            tc.tile_pool(name="redpool", bufs=2) as redpool,
            tc.tile_pool(name="epool", bufs=10) as epool,
            tc.tile_pool(name="rpool", bufs=3) as rpool,
            tc.tile_pool(name="ps_score", bufs=4, space="PSUM") as ps_score,
            tc.tile_pool(name="ps_u", bufs=2, space="PSUM") as ps_u,
            tc.tile_pool(name="ps_t", bufs=1, space="PSUM") as ps_t,
        ):
            # ---- constants first (no DMA deps; must not queue behind
            # DMA issue on any sequencer) ----
            identity = singles.tile([128, 128], F16)
            make_identity(nc, identity)
            negb = singles.tile([128, 1], F32)
            nc.vector.memset(negb, -BIAS)
            dummy = singles.tile([128, 1], F32)
            nc.scalar.activation(dummy, negb,
                                 mybir.ActivationFunctionType.Exp)
            junk = singles.tile([128, L], F16)
            nc.vector.memset(junk, 1.0)

            # ---- bulk input DMAs: few instructions (issue costs ~0.6us
            # each), fat descriptors, earliest-needed first, spread over
            # both the GpSimd (opens earlier) and SP issue paths ----
            lab_sb = singles.tile([8, L], F32)
            lab_bcast = bass.AP(tensor=lab.tensor, offset=lab.offset,
                                ap=[[0, 8]] + list(lab.ap[1:]))
            nc.gpsimd.dma_start(out=lab_sb, in_=lab_bcast)
            iota_sb = singles.tile([8, 1], F32)
            nc.gpsimd.dma_start(out=iota_sb, in_=iota8)
            qtb = singles.tile([D + 8, VH * L], F16)
            nc.gpsimd.dma_start(out=qtb[0:32, :], in_=q_t[0:32, :])
            nc.gpsimd.dma_start(out=qtb[32:D, :], in_=q_t[32:D, :])

            # key v-halves over 8 HWDGE queues (SP) — fastest measured
            # path for the 2MB key; k_a's tree overlaps k_b's transfer
            kc_a = singles.tile([128, NJ * (V // 2) * D], F16)
            kc_b = singles.tile([128, NJ * (V // 2) * D], F16)
            for i in range(4):
                nc.sync.dma_start(out=kc_a[i * 32 : (i + 1) * 32],
                                  in_=ka_in[i * 32 : (i + 1) * 32])
            for i in range(4):
                nc.sync.dma_start(out=kc_b[i * 32 : (i + 1) * 32],
                                  in_=kb_in[i * 32 : (i + 1) * 32])
            vcast = singles.tile([128, NJ, VH, D + 2], BF16)
            nc.sync.dma_start(out=vcast[0:64], in_=v_in[0:64])
            nc.sync.dma_start(out=vcast[64:128], in_=v_in[64:128])

            # one-hot label rows (device-computed from sorted labels)
            onehot = singles.tile([8, L], F32)
            nc.vector.tensor_scalar(onehot, lab_sb, iota_sb, None,
                                    op0=mybir.AluOpType.is_equal)
            oh16 = singles.tile([8, L], F16)
            nc.vector.tensor_copy(oh16, onehot)
            # replicate into the query tile rows (one 0-stride DMA)
            oh_rep = bass.AP(tensor=oh16.tensor, offset=oh16.offset,
                             ap=[list(oh16.ap[0]), [0, VH], list(oh16.ap[1])])
            nc.gpsimd.dma_start(
                out=qtb[D : D + 8, :].rearrange("p (h l) -> p h l", h=VH, l=L),
                in_=oh_rep,
            )

            # stb tiles: one-hot rows written by DVE, S^T rows by keysum
            stbs = {}
            for jc in range(NJ):
                stbs[jc] = singles.tile([D + 8, 128], F16, tag=f"stb{jc}",
                                        name=f"stb{jc}")
                nc.vector.tensor_scalar_mul(
                    stbs[jc][D : D + 8, :],
                    onehot[:, jc * 128 : (jc + 1) * 128], 8.0 * BIAS,
                )

            # ---- PE warmup so HAM reaches 2.4 GHz before the pipeline ----
            for w in range(10):
                wps = ps_u.tile([128, L], F32, tag="warm", name=f"warm{w}", bufs=1)
                nc.tensor.matmul(wps, lhsT=identity, rhs=junk,
                                 start=True, stop=True)

            # ---- keysum: each v-half reduced independently (the first
            # tree runs while the second half is still in flight), then
            # one final add ----
            def _half_tree(kc, tag_):
                width = (V // 2) * D
                cur = kc.rearrange("p (c w) -> p c w", c=NJ, w=width)
                while width > D:
                    width //= 2
                    nxt = redpool.tile([128, NJ, width], F16,
                                       tag=f"red{tag_}{width}",
                                       name=f"red{tag_}{width}")
                    nc.vector.tensor_tensor(
                        out=nxt, in0=cur[:, :, 0:width],
                        in1=cur[:, :, width : 2 * width],
                        op=mybir.AluOpType.add,
                    )
                    cur = nxt
                return cur

            s_a = _half_tree(kc_a, "a")
            s_b = _half_tree(kc_b, "b")
            cur = redpool.tile([128, NJ, D], F16, tag="redsum", name="redsum")
            nc.vector.tensor_tensor(out=cur, in0=s_a, in1=s_b,
                                    op=mybir.AluOpType.add)

            def _make_stb(jc):
                st_ps = ps_t.tile([D, 128], F16, tag="st_ps", name=f"st{jc}")
                nc.tensor.transpose(st_ps, cur[:, jc, :], identity)
                nc.vector.tensor_copy(stbs[jc][0:D, :], st_ps)

            HG = 2  # output group = 2 heads
            oc_tiles = [
                singles.tile([128, NJ, HG, D], F32, tag=f"oc{g}", name=f"oc{g}")
                for g in range(VH // HG)
            ]

            def _head_scores(h, jc):
                lo, hi = WIN[jc]
                w = 128 * (hi - lo + 1)
                ps = ps_score.tile([128, L], F32, tag="ps", name=f"ps{h}_{jc}")
                nc.tensor.matmul(
                    ps[:, 0:w], lhsT=stbs[jc],
                    rhs=qtb[:, h * L + 128 * lo : h * L + 128 * lo + w],
                    start=True, stop=True,
                )
                e_t = epool.tile([128, 3 * 128], BF16, tag="et",
                                 name=f"et{h}_{jc}")
                nc.scalar.activation(
                    e_t[:, 0:w], ps[:, 0:w], mybir.ActivationFunctionType.Exp,
                    bias=negb, scale=1.0 / 8.0,
                )
                return e_t

            def _head_tail(h, e_tiles):
                oc = oc_tiles[h // HG]
                hh = h % HG
                psu = ps_u.tile([128, NJ, D + 1], F32, tag="psu", name=f"psu{h}")
                for si in range(NJ):
                    jcs = [jc for jc in range(NJ)
                           if WIN[jc][0] <= si <= WIN[jc][1]]
                    for idx, jc in enumerate(jcs):
                        off = 128 * (si - WIN[jc][0])
                        nc.tensor.matmul(
                            psu[:, si, :],
                            lhsT=e_tiles[jc][:, off : off + 128],
                            rhs=vcast[:, jc, h, 0 : D + 1],
                            start=(idx == 0), stop=(idx == len(jcs) - 1),
                        )
                rinv = rpool.tile([128, NJ], F32, tag="rinv", name=f"rinv{h}")
                nc.vector.reciprocal(rinv, psu[:, :, D])
                rinv_b = bass.AP(
                    tensor=rinv.tensor, offset=rinv.offset,
                    ap=[list(rinv.ap[0]), list(rinv.ap[1]), [0, D]],
                )
                nc.vector.tensor_tensor(
                    out=oc[:, :, hh, :], in0=psu[:, :, 0:D], in1=rinv_b,
                    op=mybir.AluOpType.mult,
                )
                if hh == HG - 1:
                    g = h // HG
                    flat = oc.rearrange("p a b d -> p (a b d)")
                    nc.sync.dma_start(out=o_out[g, 0:64], in_=flat[0:64])
                    nc.sync.dma_start(out=o_out[g, 64:128], in_=flat[64:128])

            # ---- head pipeline. stb0/1 come up first; heads 0-1 run
            # their jc0/jc1 scores before stb2/3 exist so the PE stream
            # matches data arrival. A@V runs one head behind the scores
            # so the PE never waits on the live head's exp. ----
            _make_stb(0)
            _make_stb(1)
            e0 = {0: _head_scores(0, 0), 1: _head_scores(0, 1)}
            e1 = {0: _head_scores(1, 0), 1: _head_scores(1, 1)}
            _make_stb(2)
            _make_stb(3)
            e0[2] = _head_scores(0, 2)
            e0[3] = _head_scores(0, 3)
            e1[2] = _head_scores(1, 2)
            e1[3] = _head_scores(1, 3)
            _head_tail(0, [e0[j] for j in range(NJ)])
            prev = (1, [e1[j] for j in range(NJ)])
            for h in range(2, VH):
                e_tiles = [_head_scores(h, jc) for jc in range(NJ)]
                _head_tail(*prev)
                prev = (h, e_tiles)
            _head_tail(*prev)
    return nc


_NC_CACHE = None


def _get_nc():
    global _NC_CACHE
    if _NC_CACHE is None:
        _patch_tile_drain()
        _NC_CACHE = _build_nc()
    return _NC_CACHE


def kernel(query, key, value, label_arr):
    """Full inputs (B,L,V,D)/(B,L) -> full output (B,L,V,D)."""
    global LAST_RESULT
    import ml_dtypes
    from concourse.bass_utils import run_bass_kernel_spmd

    query = np.asarray(query, dtype=np.float32)
    key = np.asarray(key, dtype=np.float32)
    value = np.asarray(value, dtype=np.float32)
    labels = np.asarray(label_arr)
    iota = np.arange(8, dtype=np.float32).reshape(8, 1)

    in_maps, perms = [], []
    for c in range(NC):
        b, v0 = c // 2, (c % 2) * VH
        perm = np.argsort(labels[b], kind="stable")
        perms.append(perm)
        qp = query[b][perm][:, v0 : v0 + VH, :]      # (L, VH, D) sorted
        kp = key[b][perm]                            # (L, V, D) sorted
        vp_ = value[b][perm][:, v0 : v0 + VH, :]
        labp = labels[b][perm].astype(np.float32)

        vp = np.zeros((L, VH, D + 2), dtype=ml_dtypes.bfloat16)
        vp[:, :, 0:D] = vp_.astype(ml_dtypes.bfloat16)
        vp[:, :, D] = 1.0
        in_maps.append({
            "q_t": np.ascontiguousarray(qp.transpose(2, 1, 0))
                .astype(np.float16).reshape(D, VH * L),
            "ka": np.ascontiguousarray(
                kp.reshape(NJ, 128, V, D)[:, :, : V // 2]
                .transpose(1, 0, 2, 3)
            ).astype(np.float16).reshape(128, NJ * (V // 2) * D),
            "kb": np.ascontiguousarray(
                kp.reshape(NJ, 128, V, D)[:, :, V // 2 :]
                .transpose(1, 0, 2, 3)
            ).astype(np.float16).reshape(128, NJ * (V // 2) * D),
            "v": np.ascontiguousarray(
                vp.reshape(NJ, 128, VH, D + 2).transpose(1, 0, 2, 3)
            ),
            "lab": labp.reshape(1, L).copy(),
            "iota8": iota,
        })

    nc = _get_nc()
    kwargs = {}
    if PROFILE:
        kwargs["trace"] = True
    res = run_bass_kernel_spmd(nc, in_maps, list(range(NC)), **kwargs)
    LAST_RESULT = res

    out = np.empty((B, L, V, D), dtype=np.float32)
    for c in range(NC):
        b, v0 = c // 2, (c % 2) * VH
        # o: [g, p, (si hh d)] -> sorted (L, VH, D), then inverse-permute
        o = res.results[c]["o"].reshape(8, 128, NJ, HG_OUT, D)
        o_sorted = o.transpose(2, 1, 0, 3, 4).reshape(L, VH, D)
        out[b][perms[c], v0 : v0 + VH, :] = o_sorted
    return out


# revision 35
# speedup vs baseline: 1.0992x; 1.0634x over previous
"""Clustered-attention Trainium2 kernel (Bass/Tile), 8-core SPMD.

Problem (per batch b, variable k, with L=512, V=32, D=64, C=8 clusters):
    S   = sum_v key[b,:,v,:]                  # (L, D) shared key-sum
    sc  = query[b,:,k,:] @ S.T / sqrt(D)      # (L, L)
    sc  = where(label[i]==label[j], sc, -inf)
    out = softmax(sc, -1) @ value[b,:,k,:]

Sharding: 8 cores = 4 batches x 2 halves of the v axis (16 heads/core).

Key ideas (all FLOPs on device; host only reshapes/casts/permutes):
  - Host sorts each batch by label. Every cluster (<=~90 rows, hard
    bound 128 assumed) then lies within a 128-row window, so only chunk
    pairs |ci-cj| <= 1 interact: 10 of 16 (i,j) 128-chunk pairs are
    computed (62.5% of dense work on PE and ScalarE).
  - The cluster mask is folded into the scores matmul: the contraction
    dim is extended by 8 one-hot label rows scaled 8*B (B=96) on the
    lhsT side and 1.0 on the rhs side, so z = q.s + 8B*[same cluster];
    exp(z/8 - B) is exp(q.s/8) for same-cluster pairs and <= e^-61
    otherwise (vs real terms >= e^-35) -- an exact -inf mask to ~1e-10,
    with zero per-element masking cost. This also makes the sorted
    windows exact: any cross-cluster overlap inside a window is zeroed.
  - keysum via fp16 tree adds on DVE over a host-interleaved
    [128p, 4c, 2048] key layout (fat 16KB DMA descriptors), then
    PE-transposed per chunk into [S^T; 8B*onehot] lhsT tiles.
  - scores^T windows [128j, w<=384] on PE (fp16, fp32 PSUM), exp on
    ScalarE (bf16 out, zero-gap saturated -- the bottleneck engine),
    then A@V accumulates E^T as lhsT so the output lands directly as
    [i, d | denom] in PSUM; the softmax denominator comes from a ones
    column appended to V by the host. One reciprocal + one 0-stride
    broadcast multiply per head normalize it.
  - Prologue tricks: exp activation-table preloaded via a dummy op, 10
    junk matmuls warm the PE HAM clock gate to 2.4 GHz, DMA issue (~0.6
    us/dma_start, serial per sequencer) kept to few fat instructions
    split across the GpSimd and SP issue paths, A@V software-pipelined
    one head behind the scores so the PE never waits on the live exp.

Measured on trn2 (8 cores, NTFF profile): ~67 us end-to-end per core,
scale-relative absmax error ~4.7e-3 (fp16/bf16 quantization).
"""

import numpy as np

import concourse.bass as bass
import concourse.tile as tile
from concourse import mybir
from concourse.masks import make_identity
from concourse.tile import TileContext, ScopedClock

B, L, V, D = 4, 512, 32, 64
NC = 8  # cores
VH = V // 2  # heads (variables) per core
NJ = L // 128  # j/i chunks
HG_OUT = 2
BIAS = 96.0  # mask bias (see module docstring)
F32 = mybir.dt.float32
F16 = mybir.dt.float16
BF16 = mybir.dt.bfloat16

PROFILE = False  # set True from a harness to enable NTFF tracing
LAST_RESULT = None  # BassKernelResults of the most recent run

_PATCHED = False


def _patch_tile_drain():
    """Walrus on this image rejects multiple sync-waits on one instruction
    ("Too many sync wait commands"). Legalize by splitting surplus waits
    onto NoOp instructions inserted just before, on the same engine —
    identical semantics (the engine stalls at each wait in order)."""
    global _PATCHED
    if _PATCHED:
        return
    _PATCHED = True

    _orig_add = TileContext._add_instruction

    def _add_instruction(self, inst):
        si = getattr(inst, "sync_info", None)
        if (
            si is not None
            and si.on_wait
            and len(si.on_wait) > 1
            and inst.engine != mybir.EngineType.Unassigned
        ):
            waits = list(si.on_wait)
            for w in waits[:-1]:
                nop = mybir.InstNoOp(name=self.nc.get_next_instruction_name())
                nop.engine = inst.engine
                nop.sync_info = mybir.SyncInfo(on_wait=[w], on_update=[])
                _orig_add(self, nop)
            inst.sync_info = mybir.SyncInfo(
                on_wait=[waits[-1]], on_update=list(si.on_update or [])
            )
        _orig_add(self, inst)

    TileContext._add_instruction = _add_instruction

    def _drain_and_barrier(self, tick_clock, wait_clock):
        nc = self.nc
        drain_inst = nc.sync.drain()
        wait_clock.add_sem_waits(
            drain_inst.ins, ScopedClock({None: tick_clock.global_clock})
        )
        si = drain_inst.ins.sync_info
        if si is not None and si.on_wait and len(si.on_wait) > 1:
            waits = list(si.on_wait)
            drain_inst.ins.sync_info = mybir.SyncInfo(
                on_wait=waits[:1], on_update=list(si.on_update or [])
            )
            for i in range(1, len(waits)):
                nop = nc.sync.nop(nofuse=True, hint=f"drain_split_{i}")
                nop.ins.sync_info = mybir.SyncInfo(on_wait=[waits[i]], on_update=[])
        nc.all_engine_barrier()
        assert self.sems is not None
        popped = nc._tile_sem_poison_stack.pop()
        assert popped is self._sem_poison
        nc.clear_and_free_semaphores(list(self.sems.allocated().values()))
        nc.all_engine_barrier()

    TileContext._drain_and_barrier = _drain_and_barrier


def _tree_reduce_v(eng, pool, kc, tag):
    """Sum kc [128, V*D] over the v axis -> [128, D] via contiguous
    halving adds on the given engine (v-major layout: halves contiguous)."""
    cur = kc
    width = V * D
    while width > D:
        width //= 2
        nxt = pool.tile([128, width], kc.dtype, tag=f"red_{width}")
        eng.tensor_tensor(
            out=nxt, in0=cur[:, 0:width], in1=cur[:, width : 2 * width],
            op=mybir.AluOpType.add,
        )
        cur = nxt
    return cur


def _dma_split(nc, out_ap, in_ap, parts):
    """Issue `parts` dma_starts over free-dim slices so the transfer
    spreads across DMA queues instead of serializing on one."""
    w = out_ap.shape[-1]
    step = w // parts
    for i in range(parts):
        sl = slice(i * step, (i + 1) * step) if i < parts - 1 else slice(i * step, w)
        nc.sync.dma_start(out=out_ap[..., sl], in_=in_ap[..., sl])


def _build_nc():
    nc = bass.Bass("TRN2", target_bir_lowering=False, debug=False)

    # All bulk inputs arrive in 16-bit, host-prepared layouts chosen for
    # fat DMA descriptors (per-partition-contiguous rows) and are already
    # PERMUTED so labels are sorted per batch: every cluster then lives
    # inside a 128-row window, and only chunk pairs |ci-cj|<=1 interact.
    q_t = nc.dram_tensor("q_t", [D, VH * L], F16, kind="ExternalInput").ap()
    ka_in = nc.dram_tensor("ka", [128, NJ * (V // 2) * D], F16,
                           kind="ExternalInput").ap()
    kb_in = nc.dram_tensor("kb", [128, NJ * (V // 2) * D], F16,
                           kind="ExternalInput").ap()
    v_in = nc.dram_tensor("v", [128, NJ, VH, D + 2], BF16, kind="ExternalInput").ap()
    lab = nc.dram_tensor("lab", [1, L], F32, kind="ExternalInput").ap()
    iota8 = nc.dram_tensor("iota8", [8, 1], F32, kind="ExternalInput").ap()
    # output in sorted order, grouped: [g, p, si, hh, d] (4 groups x 4 heads)
    o_out = nc.dram_tensor("o", [8, 128, NJ * 2 * D], F32, kind="ExternalOutput").ap()

    # i-chunk windows per j-chunk: chunks [lo, hi] inclusive
    WIN = [(max(jc - 1, 0), min(jc + 1, NJ - 1)) for jc in range(NJ)]

    with TileContext(nc) as tc:
        with (
            tc.tile_pool(name="singles", bufs=1) as singles,
            tc.tile_pool(name="redpool", bufs=2) as redpool,
            tc.tile_pool(name="epool", bufs=10) as epool,
            tc.tile_pool(name="rpool", bufs=3) as rpool,
            tc.tile_pool(name="ps_score", bufs=4, space="PSUM") as ps_score,
            tc.tile_pool(name="ps_u", bufs=2, space="PSUM") as ps_u,
            tc.tile_pool(name="ps_t", bufs=1, space="PSUM") as ps_t,
        ):
            # ---- constants first (no DMA deps; must not queue behind
            # DMA issue on any sequencer) ----
            identity = singles.tile([128, 128], F16)
            make_identity(nc, identity)
            negb = singles.tile([128, 1], F32)
            nc.vector.memset(negb, -BIAS)
            dummy = singles.tile([128, 1], F32)
            nc.scalar.activation(dummy, negb,
                                 mybir.ActivationFunctionType.Exp)
            junk = singles.tile([128, L], F16)
            nc.vector.memset(junk, 1.0)

            # ---- bulk input DMAs: few instructions (issue costs ~0.6us
            # each), fat descriptors, earliest-needed first, spread over
            # both the GpSimd (opens earlier) and SP issue paths ----
            lab_sb = singles.tile([8, L], F32)
            lab_bcast = bass.AP(tensor=lab.tensor, offset=lab.offset,
                                ap=[[0, 8]] + list(lab.ap[1:]))
            nc.gpsimd.dma_start(out=lab_sb, in_=lab_bcast)
            iota_sb = singles.tile([8, 1], F32)
            nc.gpsimd.dma_start(out=iota_sb, in_=iota8)
            qtb = singles.tile([D + 8, VH * L], F16)
            q_dmas = [
                nc.gpsimd.dma_start(out=qtb[0:32, :], in_=q_t[0:32, :]),
                nc.gpsimd.dma_start(out=qtb[32:D, :], in_=q_t[32:D, :]),
            ]

            # key v-halves over 8 HWDGE queues (SP) — fastest measured
            # path for the 2MB key; k_a's tree overlaps k_b's transfer.
            # The HBM is bandwidth-saturated during the prologue and the
            # exp pipeline is gated by the LAST key byte, so q and v are
            # explicitly held back (dep edges) until the key halves land:
            # their own deadlines (first scores / first A@V) are later.
            kc_a = singles.tile([128, NJ * (V // 2) * D], F16)
            kc_b = singles.tile([128, NJ * (V // 2) * D], F16)
            ka_last = kb_last = None
            for i in range(4):
                ka_last = nc.sync.dma_start(
                    out=kc_a[i * 32 : (i + 1) * 32],
                    in_=ka_in[i * 32 : (i + 1) * 32])
            for i in range(4):
                kb_last = nc.sync.dma_start(
                    out=kc_b[i * 32 : (i + 1) * 32],
                    in_=kb_in[i * 32 : (i + 1) * 32])
            for qd in q_dmas:
                tile.add_dep_helper(qd.ins, ka_last.ins,
                                    reason="q waits for key half a")
            vcast = singles.tile([128, NJ, VH, D + 2], BF16)
            for sl in (slice(0, 64), slice(64, 128)):
                vd = nc.sync.dma_start(out=vcast[sl], in_=v_in[sl])
                tile.add_dep_helper(vd.ins, kb_last.ins,
                                    reason="v waits for key half b")

            # one-hot label rows (device-computed from sorted labels)
            onehot = singles.tile([8, L], F32)
            nc.vector.tensor_scalar(onehot, lab_sb, iota_sb, None,
                                    op0=mybir.AluOpType.is_equal)
            oh16 = singles.tile([8, L], F16)
            nc.vector.tensor_copy(oh16, onehot)
            # replicate into the query tile rows (one 0-stride DMA)
            oh_rep = bass.AP(tensor=oh16.tensor, offset=oh16.offset,
                             ap=[list(oh16.ap[0]), [0, VH], list(oh16.ap[1])])
            nc.gpsimd.dma_start(
                out=qtb[D : D + 8, :].rearrange("p (h l) -> p h l", h=VH, l=L),
                in_=oh_rep,
            )

            # stb tiles: one-hot rows written by DVE, S^T rows by keysum
            stbs = {}
            for jc in range(NJ):
                stbs[jc] = singles.tile([D + 8, 128], F16, tag=f"stb{jc}",
                                        name=f"stb{jc}")
                nc.vector.tensor_scalar_mul(
                    stbs[jc][D : D + 8, :],
                    onehot[:, jc * 128 : (jc + 1) * 128], 8.0 * BIAS,
                )

            # ---- PE warmup so HAM reaches 2.4 GHz before the pipeline ----
            for w in range(10):
                wps = ps_u.tile([128, L], F32, tag="warm", name=f"warm{w}", bufs=1)
                nc.tensor.matmul(wps, lhsT=identity, rhs=junk,
                                 start=True, stop=True)

            # ---- keysum: each v-half reduced independently (the first
            # tree runs while the second half is still in flight), then
            # one final add ----
            def _half_tree(kc, tag_):
                width = (V // 2) * D
                cur = kc.rearrange("p (c w) -> p c w", c=NJ, w=width)
                while width > D:
                    width //= 2
                    nxt = redpool.tile([128, NJ, width], F16,
                                       tag=f"red{tag_}{width}",
                                       name=f"red{tag_}{width}")
                    nc.vector.tensor_tensor(
                        out=nxt, in0=cur[:, :, 0:width],
                        in1=cur[:, :, width : 2 * width],
                        op=mybir.AluOpType.add,
                    )
                    cur = nxt
                return cur

            s_a = _half_tree(kc_a, "a")
            s_b = _half_tree(kc_b, "b")
            cur = redpool.tile([128, NJ, D], F16, tag="redsum", name="redsum")
            nc.vector.tensor_tensor(out=cur, in0=s_a, in1=s_b,
                                    op=mybir.AluOpType.add)

            def _make_stb(jc):
                st_ps = ps_t.tile([D, 128], F16, tag="st_ps", name=f"st{jc}")
                nc.tensor.transpose(st_ps, cur[:, jc, :], identity)
                nc.vector.tensor_copy(stbs[jc][0:D, :], st_ps)

            HG = 2  # output group = 2 heads
            oc_tiles = [
                singles.tile([128, NJ, HG, D], F32, tag=f"oc{g}", name=f"oc{g}")
                for g in range(VH // HG)
            ]

            def _head_scores(h, jc):
                lo, hi = WIN[jc]
                w = 128 * (hi - lo + 1)
                ps = ps_score.tile([128, L], F32, tag="ps", name=f"ps{h}_{jc}")
                nc.tensor.matmul(
                    ps[:, 0:w], lhsT=stbs[jc],
                    rhs=qtb[:, h * L + 128 * lo : h * L + 128 * lo + w],
                    start=True, stop=True,
                )
                e_t = epool.tile([128, 3 * 128], BF16, tag="et",
                                 name=f"et{h}_{jc}")
                nc.scalar.activation(
                    e_t[:, 0:w], ps[:, 0:w], mybir.ActivationFunctionType.Exp,
                    bias=negb, scale=1.0 / 8.0,
                )
                return e_t

            def _head_tail(h, e_tiles):
                oc = oc_tiles[h // HG]
                hh = h % HG
                psu = ps_u.tile([128, NJ, D + 1], F32, tag="psu", name=f"psu{h}")
                for si in range(NJ):
                    jcs = [jc for jc in range(NJ)
                           if WIN[jc][0] <= si <= WIN[jc][1]]
                    for idx, jc in enumerate(jcs):
                        off = 128 * (si - WIN[jc][0])
                        nc.tensor.matmul(
                            psu[:, si, :],
                            lhsT=e_tiles[jc][:, off : off + 128],
                            rhs=vcast[:, jc, h, 0 : D + 1],
                            start=(idx == 0), stop=(idx == len(jcs) - 1),
                        )
                rinv = rpool.tile([128, NJ], F32, tag="rinv", name=f"rinv{h}")
                nc.vector.reciprocal(rinv, psu[:, :, D])
                rinv_b = bass.AP(
                    tensor=rinv.tensor, offset=rinv.offset,
                    ap=[list(rinv.ap[0]), list(rinv.ap[1]), [0, D]],
                )
                nc.vector.tensor_tensor(
                    out=oc[:, :, hh, :], in0=psu[:, :, 0:D], in1=rinv_b,
                    op=mybir.AluOpType.mult,
                )
                if hh == HG - 1:
                    g = h // HG
                    flat = oc.rearrange("p a b d -> p (a b d)")
                    nc.sync.dma_start(out=o_out[g, 0:64], in_=flat[0:64])
                    nc.sync.dma_start(out=o_out[g, 64:128], in_=flat[64:128])

            # ---- head pipeline. stb0/1 come up first; heads 0-1 run
            # their jc0/jc1 scores before stb2/3 exist so the PE stream
            # matches data arrival. A@V runs one head behind the scores
            # so the PE never waits on the live head's exp. ----
            _make_stb(0)
            _make_stb(1)
            e0 = {0: _head_scores(0, 0), 1: _head_scores(0, 1)}
            e1 = {0: _head_scores(1, 0), 1: _head_scores(1, 1)}
            _make_stb(2)
            _make_stb(3)
            e0[2] = _head_scores(0, 2)
            e0[3] = _head_scores(0, 3)
            e1[2] = _head_scores(1, 2)
            e1[3] = _head_scores(1, 3)
            _head_tail(0, [e0[j] for j in range(NJ)])
            prev = (1, [e1[j] for j in range(NJ)])
            for h in range(2, VH):
                e_tiles = [_head_scores(h, jc) for jc in range(NJ)]
                _head_tail(*prev)
                prev = (h, e_tiles)
            _head_tail(*prev)
    return nc


_NC_CACHE = None


def _get_nc():
    global _NC_CACHE
    if _NC_CACHE is None:
        _patch_tile_drain()
        _NC_CACHE = _build_nc()
    return _NC_CACHE


def kernel(query, key, value, label_arr):
    """Full inputs (B,L,V,D)/(B,L) -> full output (B,L,V,D)."""
    global LAST_RESULT
    import ml_dtypes
    from concourse.bass_utils import run_bass_kernel_spmd

    query = np.asarray(query, dtype=np.float32)
    key = np.asarray(key, dtype=np.float32)
    value = np.asarray(value, dtype=np.float32)
    labels = np.asarray(label_arr)
    iota = np.arange(8, dtype=np.float32).reshape(8, 1)

    in_maps, perms = [], []
    for c in range(NC):
        b, v0 = c // 2, (c % 2) * VH
        perm = np.argsort(labels[b], kind="stable")
        perms.append(perm)
        qp = query[b][perm][:, v0 : v0 + VH, :]      # (L, VH, D) sorted
        kp = key[b][perm]                            # (L, V, D) sorted
        vp_ = value[b][perm][:, v0 : v0 + VH, :]
        labp = labels[b][perm].astype(np.float32)

        vp = np.zeros((L, VH, D + 2), dtype=ml_dtypes.bfloat16)
        vp[:, :, 0:D] = vp_.astype(ml_dtypes.bfloat16)
        vp[:, :, D] = 1.0
        in_maps.append({
            "q_t": np.ascontiguousarray(qp.transpose(2, 1, 0))
                .astype(np.float16).reshape(D, VH * L),
            "ka": np.ascontiguousarray(
                kp.reshape(NJ, 128, V, D)[:, :, : V // 2]
                .transpose(1, 0, 2, 3)
            ).astype(np.float16).reshape(128, NJ * (V // 2) * D),
            "kb": np.ascontiguousarray(
                kp.reshape(NJ, 128, V, D)[:, :, V // 2 :]
                .transpose(1, 0, 2, 3)
            ).astype(np.float16).reshape(128, NJ * (V // 2) * D),
            "v": np.ascontiguousarray(
                vp.reshape(NJ, 128, VH, D + 2).transpose(1, 0, 2, 3)
            ),
            "lab": labp.reshape(1, L).copy(),
            "iota8": iota,
        })

    nc = _get_nc()
    kwargs = {}
    if PROFILE:
        kwargs["trace"] = True
    res = run_bass_kernel_spmd(nc, in_maps, list(range(NC)), **kwargs)
    LAST_RESULT = res

    out = np.empty((B, L, V, D), dtype=np.float32)
    for c in range(NC):
        b, v0 = c // 2, (c % 2) * VH
        # o: [g, p, (si hh d)] -> sorted (L, VH, D), then inverse-permute
        o = res.results[c]["o"].reshape(8, 128, NJ, HG_OUT, D)
        o_sorted = o.transpose(2, 1, 0, 3, 4).reshape(L, VH, D)
        out[b][perms[c], v0 : v0 + VH, :] = o_sorted
    return out
